# revision 1
# baseline (speedup 1.0000x reference)
"""GCN layer relu((A_hat @ x) @ W + b) on 8 TRN2 NeuronCores (Bass/Tile).

Self-contained: kernel(**inputs) takes FULL inputs, returns FULL output.

Strategy (dst-sharded SPMD, one program on 8 cores):
  - nodes (rows of x / output) split into 8 contiguous shards of 12500.
  - per core: edges with dst in shard (plus self-loops), sorted by
    (src_chunk, dst_window); each (chunk, window) region padded to 128-slot
    blocks using the max count across cores so one SPMD program fits all.
  - src features pulled by dma_gather (SWDGE indirect DMA, int16 indices per
    25000-row chunk of x) into SBUF edge-major: slot i -> [i%128, i//128].
  - per 128-slot block: ScalarE scales rows by the edge's GCN norm
    (pads have norm=0), VectorE builds a one-hot [128,512] selection matrix
    from window-local dst ids, TensorE accumulates psum += G'.T @ S giving
    aggT [128 feat, 512 dst] per window; regions flush into an SBUF
    accumulator aggT [128, 12800].
  - phase 2 per 128-dst quarter: psum2 = aggT_slice.T @ W; +bias; relu; DMA
    to the output shard. Host concatenates shards.

Everything is fp32; the only host-side work is index/metadata preparation.
"""
import math

import numpy as np

import concourse.bacc as bacc
import concourse.mybir as mybir
import concourse.tile as tile
from concourse import bass_utils

P = 128
FEAT = 128
N_NODES = 100000
NCORES = 8
WINDOW = 512
CHUNK_ROWS = 25000
CALL_MAX_BLOCKS = 64

NPC = N_NODES // NCORES                  # 12500 dst nodes per core
NW = math.ceil(NPC / WINDOW)             # 25 windows per core
DPAD = NW * WINDOW                       # 12800 padded dst rows per core
NCHUNK = math.ceil(N_NODES / CHUNK_ROWS)  # 4 src chunks


def _host_prep(x, edge_index, edge_weight, W, b):
    src = np.asarray(edge_index[0], dtype=np.int64)
    dst = np.asarray(edge_index[1], dtype=np.int64)
    ew = np.asarray(edge_weight, dtype=np.float32)
    n = N_NODES
    loops = np.arange(n, dtype=np.int64)
    src = np.concatenate([src, loops])
    dst = np.concatenate([dst, loops])
    ew = np.concatenate([ew, np.ones(n, np.float32)])

    deg = np.bincount(dst, weights=ew.astype(np.float64), minlength=n)
    deg = deg.astype(np.float32)
    dinv = np.where(deg > 0, 1.0 / np.sqrt(deg), 0.0).astype(np.float32)
    norm = (dinv[src] * ew * dinv[dst]).astype(np.float32)

    core = dst // NPC
    dst_local = dst - core * NPC
    w_id = dst_local // WINDOW
    dst_in_w = (dst_local % WINDOW).astype(np.float32)
    c_id = src // CHUNK_ROWS
    idx_local = (src - c_id * CHUNK_ROWS).astype(np.int16)

    flat = (core * NCHUNK + c_id) * NW + w_id
    counts = np.bincount(flat, minlength=NCORES * NCHUNK * NW).reshape(
        NCORES, NCHUNK, NW)
    B = np.ceil(counts.max(axis=0) / P).astype(np.int64)  # [NCHUNK, NW]

    nb_total = int(B.sum())
    slots_total = nb_total * P

    block_base = np.zeros((NCHUNK, NW), dtype=np.int64)
    regions = []
    acc = 0
    for c in range(NCHUNK):
        for w in range(NW):
            block_base[c, w] = acc
            regions.append((c, w, acc, int(B[c, w])))
            acc += int(B[c, w])

    calls = []
    for c in range(NCHUNK):
        b0 = int(block_base[c, 0])
        b1 = int(block_base[c + 1, 0]) if c + 1 < NCHUNK else nb_total
        k = b0
        while k < b1:
            nblk = min(CALL_MAX_BLOCKS, b1 - k)
            calls.append((c, k, nblk))
            k += nblk

    meta = dict(regions=regions, calls=calls, nb_total=nb_total,
                slots_total=slots_total, B=B)

    order_all = np.lexsort((w_id, c_id, core))
    core_sorted = core[order_all]
    core_starts = np.searchsorted(core_sorted, np.arange(NCORES + 1))
    x32 = np.ascontiguousarray(np.asarray(x, dtype=np.float32))
    W32 = np.ascontiguousarray(np.asarray(W, dtype=np.float32))
    b32 = np.asarray(b, dtype=np.float32)
    iota = np.tile(np.arange(WINDOW, dtype=np.float32)[None, :], (P, 1))
    b_full = np.tile(b32[None, :], (P, 1)).astype(np.float32)

    in_maps = []
    for m in range(NCORES):
        sel = order_all[core_starts[m]:core_starts[m + 1]]
        mc, mw = c_id[sel], w_id[sel]
        midx, mdstw, mnorm = idx_local[sel], dst_in_w[sel], norm[sel]

        idx16 = np.zeros(slots_total, dtype=np.int16)
        dstloc = np.zeros(slots_total, dtype=np.float32)
        nrm = np.zeros(slots_total, dtype=np.float32)
        pos = 0
        for (c, w, blk0, nblk) in regions:
            cnt = int(counts[m, c, w])
            s0 = blk0 * P
            idx16[s0:s0 + cnt] = midx[pos:pos + cnt]
            dstloc[s0:s0 + cnt] = mdstw[pos:pos + cnt]
            nrm[s0:s0 + cnt] = mnorm[pos:pos + cnt]
            pos += cnt
        assert pos == len(sel)

        idx_tile = np.zeros((P, slots_total // 16), dtype=np.int16)
        for (c, blk0, nblk) in calls:
            s0, s1 = blk0 * P, (blk0 + nblk) * P
            seg = idx16[s0:s1].reshape(-1, 16).T
            idx_tile[:, s0 // 16:s1 // 16] = np.tile(seg, (8, 1))

        in_maps.append({
            "x": x32,
            "idx": idx_tile,
            "dstloc": dstloc.reshape(nb_total, P).T.copy(),
            "normt": nrm.reshape(nb_total, P).T.copy(),
            "iota": iota,
            "Wt": W32,
            "bfull": b_full,
        })
    return meta, in_maps


def _build_kernel(meta):
    nb_total = meta["nb_total"]
    slots_total = meta["slots_total"]
    regions = meta["regions"]
    calls = meta["calls"]
    WD = WINDOW

    nc = bacc.Bacc("TRN2", target_bir_lowering=False, debug=False,
                   num_devices=NCORES)
    x = nc.dram_tensor("x", [N_NODES, FEAT], mybir.dt.float32,
                       kind="ExternalInput")
    idx = nc.dram_tensor("idx", [P, slots_total // 16], mybir.dt.int16,
                         kind="ExternalInput")
    dstloc = nc.dram_tensor("dstloc", [P, nb_total], mybir.dt.float32,
                            kind="ExternalInput")
    normt = nc.dram_tensor("normt", [P, nb_total], mybir.dt.float32,
                           kind="ExternalInput")
    iota = nc.dram_tensor("iota", [P, WD], mybir.dt.float32,
                          kind="ExternalInput")
    Wt = nc.dram_tensor("Wt", [FEAT, FEAT], mybir.dt.float32,
                        kind="ExternalInput")
    bfull = nc.dram_tensor("bfull", [P, FEAT], mybir.dt.float32,
                           kind="ExternalInput")
    out = nc.dram_tensor("out", [DPAD, FEAT], mybir.dt.float32,
                         kind="ExternalOutput")

    Bm = meta["B"]
    first_c = {}
    for w in range(NW):
        fc = None
        for c in range(NCHUNK):
            if Bm[c, w] > 0:
                fc = c
                break
        first_c[w] = fc

    with tile.TileContext(nc) as tc:
        with (
            tc.tile_pool(name="const", bufs=1) as constp,
            tc.tile_pool(name="agg", bufs=1) as aggp,
            tc.tile_pool(name="gbuf", bufs=2) as gbufp,
            tc.tile_pool(name="gsc", bufs=6) as gscp,
            tc.tile_pool(name="sel", bufs=6) as selp,
            tc.tile_pool(name="ps1", bufs=2, space="PSUM") as ps1p,
            tc.tile_pool(name="ps2", bufs=2, space="PSUM") as ps2p,
            tc.tile_pool(name="outst", bufs=3) as outp,
        ):
            idx_sb = constp.tile([P, slots_total // 16], mybir.dt.int16)
            dstloc_sb = constp.tile([P, nb_total], mybir.dt.float32)
            normt_sb = constp.tile([P, nb_total], mybir.dt.float32)
            iota_sb = constp.tile([P, WD], mybir.dt.float32)
            W_sb = constp.tile([FEAT, FEAT], mybir.dt.float32)
            b_sb = constp.tile([P, FEAT], mybir.dt.float32)
            aggT = aggp.tile([P, DPAD], mybir.dt.float32)

            nc.sync.dma_start(out=idx_sb[:], in_=idx[:])
            nc.sync.dma_start(out=dstloc_sb[:], in_=dstloc[:])
            nc.sync.dma_start(out=normt_sb[:], in_=normt[:])
            nc.sync.dma_start(out=iota_sb[:], in_=iota[:])
            nc.sync.dma_start(out=W_sb[:], in_=Wt[:])
            nc.sync.dma_start(out=b_sb[:], in_=bfull[:])

            gtiles = {}
            issued = set()
            call_of_block = {}
            for ci, (c, blk0, nblk) in enumerate(calls):
                for bb in range(blk0, blk0 + nblk):
                    call_of_block[bb] = ci

            def gather_call(ci):
                c, blk0, nblk = calls[ci]
                g = gbufp.tile([P, CALL_MAX_BLOCKS, FEAT], mybir.dt.float32,
                               tag="g")
                nidx = nblk * P
                nc.gpsimd.dma_gather(
                    g[:, :nblk, :],
                    x[c * CHUNK_ROWS:min((c + 1) * CHUNK_ROWS, N_NODES), :],
                    idx_sb[:, blk0 * 8:(blk0 + nblk) * 8],
                    nidx, nidx, FEAT, single_packet=False,
                )
                gtiles[ci] = (g, blk0, nblk)

            for (c, w, blk0, nblk) in regions:
                if nblk == 0:
                    continue
                ps = ps1p.tile([P, WD], mybir.dt.float32, tag="ps1")
                for j, gb in enumerate(range(blk0, blk0 + nblk)):
                    ci = call_of_block[gb]
                    if ci not in issued:
                        gather_call(ci)
                        issued.add(ci)
                    g, cblk0, cnblk = gtiles[ci]
                    col = gb - cblk0
                    gsc = gscp.tile([P, FEAT], mybir.dt.float32, tag="gsc")
                    nc.scalar.activation(
                        gsc[:], g[:, col, :],
                        mybir.ActivationFunctionType.Copy,
                        scale=normt_sb[:, gb:gb + 1],
                    )
                    st = selp.tile([P, WD], mybir.dt.float32, tag="sel")
                    nc.vector.tensor_tensor(
                        out=st[:],
                        in0=iota_sb[:],
                        in1=dstloc_sb[:, gb:gb + 1].to_broadcast([P, WD]),
                        op=mybir.AluOpType.is_equal,
                    )
                    nc.tensor.matmul(
                        out=ps[:], lhsT=gsc[:], rhs=st[:],
                        start=(j == 0), stop=(j == nblk - 1),
                    )
                wsl = aggT[:, w * WD:(w + 1) * WD]
                if c == first_c[w]:
                    nc.vector.tensor_copy(out=wsl, in_=ps[:])
                else:
                    nc.vector.tensor_add(out=wsl, in0=wsl, in1=ps[:])

            for w in range(NW):
                if first_c[w] is None:
                    nc.vector.memset(aggT[:, w * WD:(w + 1) * WD], 0.0)

            for w in range(NW):
                for q in range(WD // P):
                    d0 = w * WD + q * P
                    ps2 = ps2p.tile([P, FEAT], mybir.dt.float32, tag="ps2")
                    nc.tensor.matmul(
                        out=ps2[:], lhsT=aggT[:, d0:d0 + P], rhs=W_sb[:],
                        start=True, stop=True,
                    )
                    ot = outp.tile([P, FEAT], mybir.dt.float32, tag="ot")
                    nc.vector.tensor_add(out=ot[:], in0=ps2[:], in1=b_sb[:])
                    nc.vector.tensor_scalar_max(ot[:], ot[:], 0.0)
                    nc.sync.dma_start(out=out[d0:d0 + P, :], in_=ot[:])
    nc.compile()
    return nc


def kernel(x, edge_index, edge_weight, W, b):
    assert x.shape == (N_NODES, FEAT)
    meta, in_maps = _host_prep(x, edge_index, edge_weight, W, b)
    nc = _build_kernel(meta)
    res = bass_utils.run_bass_kernel_spmd(
        nc, in_maps, core_ids=list(range(NCORES)), trace=False)
    outs = [res.results[m]["out"][:NPC] for m in range(NCORES)]
    return np.ascontiguousarray(np.concatenate(outs, axis=0))

